# revision 32
# baseline (speedup 1.0000x reference)
"""CPPN MLP (12 -> 32 -> 32 -> 32 -> 3, per-node activations) on 8 TRN2 cores.

Data-parallel over the pixel axis. Each core processes P_CORE pixels laid out
feature-major as 4 pixel-groups on SBUF partitions:
  rhs partition (12*g + i) holds feature i of pixel-group g  (layer-1 input)
  hidden state partition layout per layer: 4 groups x 32 nodes, nodes sorted
  [sin | gauss | tanh-class] across groups so activation passes are prefix
  ranges starting at partition 0 (ISA requires start partition in {0,32,64,96}).

Matmuls use permuted block-diagonal stationary matrices (float32r = full-rate
fp32).  Per-node activation selection is done with per-partition scale/bias
operand columns on the ScalarE activation instruction plus host-side algebraic
folds into the next layer's weights:
  sigmoid(z) = 0.5*tanh(z/2) + 0.5          (stored tanh(z/2); affine folded)
  identity(z) = tanh(eps*z)/eps             (stored tanh(eps*z); 1/eps folded)
  gauss(z) = exp(-z^2/2) = (1-t)/(1+t),  t = tanh(z^2/4)   (Square+Tanh+DVE)
  sin(z): k = round(z/2pi) via fp32 magic-constant rounding (two Identity
  passes), Cody-Waite 3-term reduction on DVE, then the Sin table (+-pi domain).
All five per-node functions resolve to {Tanh, Square, Identity, Sin} which
co-reside in one activation table set (no table switching).
"""

import os
import sys

import numpy as np

_REPO = "/root/.axon_site/_ro/trn_rl_repo"
if _REPO not in sys.path and not os.path.isdir("/opt/trn_rl_repo"):
    sys.path.insert(0, _REPO)

import concourse.bacc as bacc
import concourse.bass as bass  # noqa: F401
import concourse.tile as tile
from concourse import mybir
from concourse.bass_utils import run_bass_kernel_spmd

# Pin the activation-function table to the single set containing every
# function this kernel uses ({Tanh, Square, Identity, Sin}).  Without this,
# bacc's greedy per-instruction set selection alternates between sets (Sin
# lives only in the trig/silu sets) and emits an ACT_TABLE_LOAD (~1.3us)
# per chunk.
_orig_get_tables = bacc.get_activation_tables


def _pinned_tables(arch):
    t = _orig_get_tables(arch)
    if "silu_and_others" in t:
        # act_func_set_id is the POSITION in act_info.json's set list, so
        # keep every entry (order intact) and just empty the others.
        return {name: (funcs if name == "silu_and_others" else set())
                for name, funcs in t.items()}
    return t


bacc.get_activation_tables = _pinned_tables

F32 = mybir.dt.float32
F32R = mybir.dt.float32r

P_TOTAL = 1024 * 1024
N_IN, H, N_OUT = 12, 32, 3
N_CORES = 8
P_CORE = P_TOTAL // N_CORES  # 131072
G = 4                        # pixel groups packed on partitions
PG = P_CORE // G             # 32768 pixels per group per core
CHUNK = 1024                 # pixels per group per chunk (2 PSUM banks)
MM_N = 512                   # matmul moving free dim (one PSUM bank)
MAGIC = np.float32(1.5 * 2 ** 23)   # fp32 round-to-nearest-int magic constant
INV_2PI = np.float32(1.0 / (2.0 * np.pi))
ID_EPS = np.float32(2.0 ** -18)     # identity-via-tanh input scale

# Cody-Waite split of 2*pi into 3 fp32 terms (computed in fp64)
_2PI = 2.0 * np.pi
CW1 = np.float32(_2PI)
CW2 = np.float32(_2PI - float(CW1))
CW3 = np.float32(_2PI - float(CW1) - float(CW2))

# class codes: 0 = sin, 1 = gauss, 2 = tanh-class (tanh/sigmoid/identity)
def _cls_of_act(a):
    return {3: 0, 4: 1}.get(int(a), 2)


def _sorted_layout(act):
    """Order the H nodes by [sin | gauss | rest]; return (perm, n_sin, n_gauss).
    perm[j] = original node index placed at sorted slot j."""
    cls = np.array([_cls_of_act(a) for a in act])
    perm = np.argsort(cls, kind="stable")
    return perm, int((cls == 0).sum()), int((cls == 1).sum())


class _Plan:
    """Host-side folded weights + per-layer layouts. All float64 math."""

    def __init__(self, bias_in, W1, b1, act1, W2, b2, act2, W3, b3, act3,
                 Wout, bout):
        layers = [(W1, b1, act1), (W2, b2, act2), (W3, b3, act3)]
        self.perms, self.nsin, self.ngauss = [], [], []
        self.lhsT = []          # device stationary matrices (np.float32)
        self.cols = []          # per-layer dict of [128] operand columns
        # incoming per-node output transform: h_true = alpha*stored + beta
        in_alpha = np.ones(N_IN, dtype=np.float64)
        in_beta = np.asarray(bias_in, dtype=np.float64)  # h0 = x + bias_in
        in_dim = N_IN
        in_layout = None  # for L1 the input layout is the fixed feature order

        for li, (W, b, act) in enumerate(layers):
            W = np.asarray(W, dtype=np.float64)
            b = np.asarray(b, dtype=np.float64)
            act = np.asarray(act)
            perm, ns, ng = _sorted_layout(act)
            self.perms.append(perm)
            self.nsin.append(ns)
            self.ngauss.append(ng)

            # effective weights / bias absorbing incoming transforms
            W_eff = W * in_alpha[:, None]                  # [in_dim, H]
            b_eff = b + in_beta @ W                        # [H]

            # device stationary: block diagonal over groups with node sort
            K = G * in_dim
            lt = np.zeros((K, 128), dtype=np.float64)
            for g in range(G):
                for j in range(H):
                    node = perm[j]
                    m = self._row(li, g, j)
                    if li == 0:
                        rows = np.arange(in_dim) + in_dim * g
                        lt[rows, m] = W_eff[:, node]
                    else:
                        for k_in in range(in_dim):
                            kpart = in_layout[g][k_in]
                            lt[kpart, m] = W_eff[k_in, node]
            self.lhsT.append(lt.astype(np.float32))

            # activation operand columns, indexed by device partition
            tanh_scale = np.zeros(128, dtype=np.float64)
            tanh_bias = np.zeros(128, dtype=np.float64)
            sq_scale = np.zeros(128, dtype=np.float64)
            sq_bias = np.zeros(128, dtype=np.float64)
            p1_bias = np.full(128, float(MAGIC), dtype=np.float64)
            sin_bias = np.zeros(128, dtype=np.float64)
            out_alpha = np.ones(H, dtype=np.float64)
            out_beta = np.zeros(H, dtype=np.float64)
            for j in range(H):
                node = perm[j]
                a = int(act[node])
                be = b_eff[node]
                for g in range(G):
                    m = self._row(li, g, j)
                    if a == 1:        # tanh
                        tanh_scale[m] = 1.0
                        tanh_bias[m] = be
                    elif a == 2:      # sigmoid -> tanh(z/2)
                        tanh_scale[m] = 0.5
                        tanh_bias[m] = 0.5 * be
                    elif a == 0:      # identity -> tanh(eps*z)
                        tanh_scale[m] = float(ID_EPS)
                        tanh_bias[m] = float(ID_EPS) * be
                    elif a == 3:      # sin
                        sin_bias[m] = be
                    elif a == 4:      # gauss: y=(z/2)^2 then tanh
                        sq_scale[m] = 0.5
                        sq_bias[m] = 0.5 * be
                if a == 1:
                    out_alpha[node], out_beta[node] = 1.0, 0.0
                elif a == 2:
                    out_alpha[node], out_beta[node] = 0.5, 0.5
                elif a == 0:
                    out_alpha[node], out_beta[node] = 1.0 / float(ID_EPS), 0.0
                elif a == 3:
                    out_alpha[node], out_beta[node] = 1.0, 0.0
                elif a == 4:
                    out_alpha[node], out_beta[node] = 1.0, 0.0
            self.cols.append({
                "tanh_scale": tanh_scale, "tanh_bias": tanh_bias,
                "sq_scale": sq_scale, "sq_bias": sq_bias,
                "p1_bias": p1_bias, "sin_bias": sin_bias,
            })

            # next layer's incoming transform, in SORTED node order per device
            # partition -> but folds are per node; store per-node arrays and
            # the partition layout for the next lhsT build.
            in_alpha = out_alpha
            in_beta = out_beta
            in_dim = H
            # partition index of (g, sorted-slot j) for this layer's output
            in_layout = [[self._row(li, g, j) for j in range(H)]
                         for g in range(G)]
            # reorder alpha/beta to sorted-slot order for the next W_eff
            in_alpha = out_alpha[perm]
            in_beta = out_beta[perm]
            # next layer's W rows must be permuted accordingly
            if li < 2:
                layers[li + 1] = (np.asarray(layers[li + 1][0])[perm, :],
                                  layers[li + 1][1], layers[li + 1][2])
            else:
                self._wout_perm = perm

        # output layer
        Wo = np.asarray(Wout, dtype=np.float64)[self._wout_perm, :]
        bo = np.asarray(bout, dtype=np.float64)
        Wo_eff = Wo * in_alpha[:, None]
        bo_eff = bo + in_beta @ Wo
        lt = np.zeros((128, 32), dtype=np.float64)
        for g in range(G):
            for j in range(H):
                kpart = in_layout[g][j]
                for o in range(N_OUT):
                    lt[kpart, 3 * g + o] = Wo_eff[j, o]
        self.lhsT_out = lt.astype(np.float32)
        out_bias = np.zeros(128, dtype=np.float64)
        for q in range(4):
            for g in range(G):
                for o in range(N_OUT):
                    out_bias[32 * q + 3 * g + o] = bo_eff[o]
        self.out_bias = out_bias

        # pack all operand columns into one [128, 32] block
        colblk = np.zeros((128, 32), dtype=np.float64)
        for li in range(3):
            c = self.cols[li]
            colblk[:, 8 * li + 0] = c["tanh_scale"]
            colblk[:, 8 * li + 1] = c["tanh_bias"]
            colblk[:, 8 * li + 2] = c["sq_scale"]
            colblk[:, 8 * li + 3] = c["sq_bias"]
            colblk[:, 8 * li + 4] = c["p1_bias"]
            colblk[:, 8 * li + 5] = c["sin_bias"]
        colblk[:, 24] = self.out_bias
        colblk[:, 25] = -float(MAGIC)
        colblk[:, 26] = float(INV_2PI)
        colblk[:, 27] = float(MAGIC)
        self.colblk = colblk.astype(np.float32)

    @staticmethod
    def _row(li, g, j):
        """Device partition of sorted-slot j, group g (layer output layout).
        Rows are class-sorted ACROSS groups: slot j occupies partitions
        4*j + g."""
        return 4 * j + g

    def prefix_sizes(self, li):
        ns, ng = self.nsin[li], self.ngauss[li]
        return 4 * ns, 4 * (ns + ng)


def _build_program(nsin, ngauss, p_core=P_CORE, chunk=CHUNK,
                   use_fp32r=False):
    """Build the bass module. Program structure depends only on the per-layer
    (n_sin, n_gauss) counts (prefix range lengths), not on weight values."""
    pg = p_core // G
    nchunk = pg // chunk
    nhalf = chunk // MM_N
    assert chunk % MM_N == 0 and pg % chunk == 0

    nc = bacc.Bacc("TRN2", target_bir_lowering=False, debug=False,
                   num_devices=N_CORES)
    xT = nc.dram_tensor("xT", [G * N_IN, pg], F32, kind="ExternalInput").ap()
    cst = nc.dram_tensor("cst", [128, 480], F32, kind="ExternalInput").ap()
    yT = nc.dram_tensor("yT", [12, pg], F32, kind="ExternalOutput").ap()

    with tile.TileContext(nc) as tc:
        cpool = tc.alloc_tile_pool(name="consts", bufs=1)
        wdt = F32R if use_fp32r else F32
        wst_t = cpool.tile([128, 416], wdt, tag="wst")
        cc_t = cpool.tile([128, 64], F32, tag="cc")
        if use_fp32r:
            nc.gpsimd.dma_start(out=wst_t[:], in_=cst[:, 0:416])
        else:
            nc.sync.dma_start(out=wst_t[:], in_=cst[:, 0:416])
        nc.sync.dma_start(out=cc_t[:], in_=cst[:, 416:480])
        w1_t = wst_t[:, 0:128]
        w2_t = wst_t[:, 128:256]
        w3_t = wst_t[:, 256:384]
        wo_t = wst_t[:, 384:416]
        col_t = cc_t[:, 0:32]

        xpool = tc.alloc_tile_pool(name="xin", bufs=4)
        hpool = tc.alloc_tile_pool(name="h", bufs=8)
        spool = tc.alloc_tile_pool(name="scratch", bufs=3)
        opool = tc.alloc_tile_pool(name="osb", bufs=2)
        ppool = tc.alloc_tile_pool(name="psum", bufs=3, space="PSUM")
        oppool = tc.alloc_tile_pool(name="psum_o", bufs=2, space="PSUM")

        w_tiles = [w1_t, w2_t, w3_t]
        osb = None
        for c in range(nchunk):
            x_t = xpool.tile([G * N_IN, chunk], F32R if use_fp32r else F32,
                             tag="x")
            if use_fp32r:
                nc.gpsimd.dma_start(
                    out=x_t[:], in_=xT[:, c * chunk:(c + 1) * chunk])
            else:
                nc.sync.dma_start(
                    out=x_t[:], in_=xT[:, c * chunk:(c + 1) * chunk])

            h_prev = x_t
            for li in range(3):
                pref_s, pref_sg = 4 * nsin[li], 4 * (nsin[li] + ngauss[li])
                kdim = G * N_IN if li == 0 else 128
                ps = ppool.tile([128, chunk], F32, tag="pre")
                wt = w_tiles[li]
                for hh in range(nhalf):
                    sl = slice(hh * MM_N, (hh + 1) * MM_N)
                    nc.tensor.matmul(
                        ps[:, sl],
                        wt[0:kdim, :],
                        h_prev[0:kdim, sl],
                        start=True, stop=True,
                    )
                h = hpool.tile([128, chunk], F32R if use_fp32r else F32,
                               tag="h")
                cb = 8 * li
                # 1) tanh-class over all 128 rows (junk on sin/gauss rows)
                nc.scalar.activation(
                    h[:], ps[:], mybir.ActivationFunctionType.Tanh,
                    bias=col_t[:, cb + 1:cb + 2],
                    scale=col_t[:, cb + 0:cb + 1],
                )
                if pref_sg > pref_s:
                    # 2) gauss: y = ((z)/2)^2 ; t = tanh(y);
                    #    h = (1-t)/(1+t) = exp(-z^2/2)
                    y_t = spool.tile([128, chunk], F32, tag="sq")
                    nc.scalar.activation(
                        y_t[0:pref_sg, :], ps[0:pref_sg, :],
                        mybir.ActivationFunctionType.Square,
                        bias=col_t[0:pref_sg, cb + 3:cb + 4],
                        scale=col_t[0:pref_sg, cb + 2:cb + 3],
                    )
                    t_t = spool.tile([128, chunk], F32, tag="tg")
                    nc.scalar.activation(
                        t_t[0:pref_sg, :], y_t[0:pref_sg, :],
                        mybir.ActivationFunctionType.Tanh,
                    )
                    num_t = spool.tile([128, chunk], F32, tag="num")
                    den_t = spool.tile([128, chunk], F32, tag="den")
                    nc.vector.tensor_scalar(
                        num_t[0:pref_sg, :], t_t[0:pref_sg, :],
                        -1.0, 1.0, mybir.AluOpType.mult, mybir.AluOpType.add)
                    nc.vector.tensor_scalar(
                        den_t[0:pref_sg, :], t_t[0:pref_sg, :],
                        1.0, 1.0, mybir.AluOpType.mult, mybir.AluOpType.add)
                    rin_t = spool.tile([128, chunk], F32, tag="rin")
                    rsc_t = spool.tile([128, chunk], F32, tag="rsc")
                    nc.vector.reciprocal_approx_accurate(
                        rin_t[0:pref_sg, :], den_t[0:pref_sg, :],
                        rsc_t[0:pref_sg, :])
                    nc.vector.tensor_tensor(
                        h[0:pref_sg, :], num_t[0:pref_sg, :],
                        rin_t[0:pref_sg, :], mybir.AluOpType.mult)
                if pref_s > 0:
                    # 3) sin with range reduction to [-pi, pi]:
                    #    t0 = z = u + b;  k = round(z/2pi) via magic const;
                    #    ur = z - 2pi*k (Cody-Waite);  h = Sin(ur)
                    t0 = spool.tile([128, chunk], F32, tag="t0")
                    nc.vector.tensor_scalar(
                        t0[0:pref_s, :], ps[0:pref_s, :],
                        col_t[0:pref_s, cb + 5:cb + 6], None,
                        mybir.AluOpType.add)
                    t1 = spool.tile([128, chunk], F32, tag="t1")
                    nc.scalar.activation(
                        t1[0:pref_s, :], t0[0:pref_s, :],
                        mybir.ActivationFunctionType.Identity,
                        bias=col_t[0:pref_s, 27:28],
                        scale=col_t[0:pref_s, 26:27],
                    )
                    kr = spool.tile([128, chunk], F32, tag="kr")
                    nc.scalar.activation(
                        kr[0:pref_s, :], t1[0:pref_s, :],
                        mybir.ActivationFunctionType.Identity,
                        bias=col_t[0:pref_s, 25:26],
                    )
                    ur = spool.tile([128, chunk], F32, tag="ur")
                    nc.vector.cody_waite_cascade(
                        ur[0:pref_s, :], t0[0:pref_s, :], kr[0:pref_s, :],
                        float(CW1), float(CW2), float(CW3))
                    nc.scalar.activation(
                        h[0:pref_s, :], ur[0:pref_s, :],
                        mybir.ActivationFunctionType.Sin,
                    )
                h_prev = h

            # output layer: quadrant-packed [12,512] matmuls
            q0 = 2 * (c % 2)
            if q0 == 0:
                pso = oppool.tile([128, MM_N], F32, tag="preo")
            for hh in range(nhalf):
                q = q0 + hh
                # fp32r forbids nonzero column tile_position: run the small
                # output-layer matmuls in plain fp32 (bitcast is free).
                nc.tensor.matmul(
                    pso[32 * q:32 * q + 32, :],
                    wo_t.bitcast(F32) if use_fp32r else wo_t,
                    h_prev[:, hh * MM_N:(hh + 1) * MM_N].bitcast(F32)
                    if use_fp32r
                    else h_prev[:, hh * MM_N:(hh + 1) * MM_N],
                    start=True, stop=True,
                    tile_position=(0, 32 * q),
                )
            if q0 == 2:
                osb = opool.tile([128, MM_N], F32, tag="osb")
                nc.scalar.activation(
                    osb[:], pso[:],
                    mybir.ActivationFunctionType.Tanh,
                    bias=col_t[:, 24:25],
                )
                base = (c - 1) * chunk
                for q in range(4):
                    nc.sync.dma_start(
                        out=yT[:, base + q * MM_N: base + (q + 1) * MM_N],
                        in_=osb[32 * q:32 * q + 12, :])

        for p in (oppool, ppool, opool, spool, hpool, xpool, cpool):
            p.release()

    nc.compile()
    return nc


_PROGRAM_CACHE = {}


def _get_program(nsin, ngauss, p_core=P_CORE, chunk=CHUNK, use_fp32r=False):
    key = (tuple(nsin), tuple(ngauss), p_core, chunk, use_fp32r)
    if key not in _PROGRAM_CACHE:
        _PROGRAM_CACHE[key] = _build_program(nsin, ngauss, p_core, chunk,
                                             use_fp32r=use_fp32r)
    return _PROGRAM_CACHE[key]


def make_in_maps(inputs, plan, p_core=P_CORE, n_cores=N_CORES):
    """Shard + transpose the pixel data; replicate constants."""
    x = np.ascontiguousarray(np.asarray(inputs["inputs"], dtype=np.float32))
    pg = p_core // G
    in_maps = []
    for core in range(n_cores):
        xc = x[core * p_core:(core + 1) * p_core]          # [p_core, 12]
        xg = xc.reshape(G, pg, N_IN)                        # [G, pg, 12]
        xT = np.ascontiguousarray(
            xg.transpose(0, 2, 1).reshape(G * N_IN, pg))    # [48, pg]
        cst = np.zeros((128, 480), dtype=np.float32)
        cst[0:G * N_IN, 0:128] = plan.lhsT[0]
        cst[:, 128:256] = plan.lhsT[1]
        cst[:, 256:384] = plan.lhsT[2]
        cst[:, 384:416] = plan.lhsT_out
        cst[:, 416:448] = plan.colblk
        in_maps.append({"xT": xT, "cst": cst})
    return in_maps


def assemble_output(results, p_core=P_CORE, n_cores=N_CORES):
    pg = p_core // G
    out = np.empty((p_core * n_cores, N_OUT), dtype=np.float32)
    for core in range(n_cores):
        yT = results[core]["yT"]                            # [12, pg]
        yc = yT.reshape(G, N_OUT, pg).transpose(0, 2, 1)    # [G, pg, 3]
        out[core * p_core:(core + 1) * p_core] = yc.reshape(p_core, N_OUT)
    return out


def make_plan(inputs):
    return _Plan(
        inputs["bias_in"], inputs["W1"], inputs["b1"], inputs["act1"],
        inputs["W2"], inputs["b2"], inputs["act2"],
        inputs["W3"], inputs["b3"], inputs["act3"],
        inputs["Wout"], inputs["bout"])


def run(inputs, trace=False, use_fp32r=False, **spmd_kwargs):
    plan = make_plan(inputs)
    nc = _get_program(plan.nsin, plan.ngauss, use_fp32r=use_fp32r)
    in_maps = make_in_maps(inputs, plan)
    res = run_bass_kernel_spmd(nc, in_maps, list(range(N_CORES)),
                               trace=trace, **spmd_kwargs)
    return assemble_output(res.results), res


def kernel(**inputs) -> np.ndarray:
    out, _ = run(inputs, trace=False)
    return out
